# revision 14
# baseline (speedup 1.0000x reference)
"""ROI-Align + MLP classification head (nms_detection) on 8 Trainium2 cores.

Strategy: data-parallel over batch (2 images per core). Per core, the kernel
computes bilinear sample coordinates from the proposals on-device, gathers
only the needed feature-map pixel pairs with indirect DMAs (~3 MB instead
of streaming the full 32 MB shard), does the bilinear combine on the vector
engine, transposes sample-major -> feature-major on the PE, and runs the
3-layer MLP + softmax.

Layouts (per core): 44 rois x 16 bin-centers = 704 samples.
  roi slot (h, g): roi = h*6 + g, h in 0..7, g in 0..5 (48 slots, 4 dup/garbage)
  sample partition p = h*16 + q (q = iy*4+ix), sample group = g.
  gather block j = ab*6 + g (ab = y-corner row 0/1), one indirect DMA each:
    G[p, j*512 :+512] = fm row pair (y0+ab, x0..x0+1) channels (512 floats).
"""

import numpy as np

import concourse.bacc as bacc
import concourse.bass as bass
import concourse.mybir as mybir
import concourse.tile as tile
from concourse._compat import get_trn_type
from concourse.bass_utils import run_bass_kernel_spmd

# Problem shape (hardcoded per contract)
B, P, H, W, C = 16, 22, 128, 128, 256
NUM_CLASSES = 10
N_CORES = 8
B_LOC = B // N_CORES        # 2 images per core
NROI = B_LOC * P            # 44 rois per core
NRS = 48                    # roi slots (8 partition-blocks x 6 groups)
NG = 6                      # sample groups of 128
HID1, HID2 = 128, 64
F32 = mybir.dt.float32
I32 = mybir.dt.int32
AX_X = mybir.AxisListType.X
OP = mybir.AluOpType
AF = mybir.ActivationFunctionType

NPIX = B_LOC * H * W            # 32768 flat pixel rows per core
MAX_ROW_A = NPIX - 130          # room for +1 col pair and +W row
MAGIC = 12582912.0              # 1.5 * 2^23 fp32 round-to-int magic


def _static_consts():
    ident = np.eye(128, dtype=np.float32)
    p = np.arange(128)
    q = p % 16
    cy = ((q // 4).astype(np.float32) + 0.5) / 4.0
    cx = ((q % 4).astype(np.float32) + 0.5) / 4.0
    cycx = np.stack([cy, cx], axis=1).astype(np.float32)          # [128, 2]
    # per-sample batch offset: bofs[p, g] for roi = (p//16)*6 + g
    h = np.arange(128)[:, None] // 16
    g = np.arange(NG)[None, :]
    roi = h * 6 + g                                               # [128, 6]
    bofs = np.where(roi >= P, float(H * W), 0.0).astype(np.float32)
    return ident, cycx, bofs


def emit_kernel(nc, tc, fm, prop, W1, b1, W2, b2, W3, b3, out, consts):
    """Emit the per-core tile kernel. All args are bass.APs."""
    with (
        tc.tile_pool(name="const", bufs=1) as cpool,
        tc.tile_pool(name="work", bufs=1) as wpool,
        tc.tile_pool(name="psum", bufs=1, space="PSUM") as ppool,
    ):
        _emit_body(nc, tc, fm, prop, W1, b1, W2, b2, W3, b3, out, consts,
                   cpool, wpool, ppool)


def _coords_expand(nc, V, prop, Ct, order):
    """Broadcast-replicate proposals into per-partition layout.

    order='gk': Ct[p, g*4+k] = prop[roi(p//16, g), k]          ([128, 24])
    order='gh': Ct[p, g*32 + h*4 + k] = prop[roi(h, g), k]     ([128, 192])
    roi(h, g) = h*6 + g, with the h=7 block read from roi 38 (clamped) and
    fixed up so (h=7, g=0,1) hold rois 42, 43.
    """
    pv = prop.rearrange("b p k -> (b p k)")                        # [176]
    for h in range(8):
        start = min(h * 6, 38) * 4
        src = pv[start:start + 24].unsqueeze(0)                    # [1, 24]
        if order == "gk":
            dst = Ct[h * 16:(h + 1) * 16, 0:24].rearrange("p (g k) -> p g k", g=6)
            nc.sync.dma_start(dst, src.rearrange("o (g k) -> o g k", g=6)
                              .to_broadcast([16, 6, 4]))
        else:
            dst = Ct[:, :].rearrange("p (g h k) -> p g h k", g=6, h=8)[:, :, h, :]
            nc.sync.dma_start(dst, src.rearrange("o (g k) -> o g k", g=6)
                              .to_broadcast([128, 6, 4]))
    # h=7 fixup: slots (g=0,1) must hold rois 42,43 (read rois 38..43)
    if order == "gk":
        nc.sync.dma_start(Ct[112:128, 0:8], Ct[112:128, 16:24])
    else:
        vw = Ct[:, :].rearrange("p (g h k) -> p g h k", g=6, h=8)
        V.tensor_copy(out=vw[:, 0:2, 7, :], in_=vw[:, 4:6, 7, :])


def _emit_body(nc, tc, fm, prop, W1, b1, W2, b2, W3, b3, out, consts,
               cpool, wpool, ppool):
    ident_c, cycx_c, bofs_c = consts
    V = nc.vector

    # ---------------- constant / weight loads ----------------
    ident = cpool.tile([128, 128], F32, name="ident")
    nc.sync.dma_start(ident[:], ident_c)
    cycx = cpool.tile([128, 2], F32, name="cycx")
    nc.sync.dma_start(cycx[:], cycx_c)
    bofs = cpool.tile([128, NG], F32, name="bofs")
    nc.sync.dma_start(bofs[:], bofs_c)

    W1sb = cpool.tile([128, 4096], F32, name="W1sb")
    nc.sync.dma_start(W1sb[:, :].rearrange("p (k h) -> p k h", k=32),
                      W1.rearrange("(k p) h -> p k h", p=128))
    W2sb = cpool.tile([128, HID2], F32, name="W2sb")
    nc.sync.dma_start(W2sb[:], W2)
    W3sb = cpool.tile([HID2, NUM_CLASSES], F32, name="W3sb")
    nc.sync.dma_start(W3sb[:], W3)
    b1sb = cpool.tile([128, 1], F32, name="b1sb")
    nc.sync.dma_start(b1sb[:], b1.rearrange("(p o) -> p o", o=1))
    b2sb = cpool.tile([HID2, 1], F32, name="b2sb")
    nc.sync.dma_start(b2sb[:], b2.rearrange("(p o) -> p o", o=1))
    b3sb = cpool.tile([NROI, NUM_CLASSES], F32, name="b3sb")
    nc.sync.dma_start(b3sb[:], b3.unsqueeze(0).to_broadcast([NROI, NUM_CLASSES]))

    # ============ coords + weights + indices (per-sample layout) ============
    Ct = cpool.tile([128, NG * 4], F32, name="coords")
    _coords_expand(nc, V, prop, Ct, "gk")
    cgv = Ct[:, :].rearrange("p (g k) -> p g k", g=NG)
    y1c, x1c, y2c, x2c = (cgv[:, :, k] for k in range(4))

    def t6(name):
        return wpool.tile([128, NG], F32, name=name)

    dy, ys, dx, xs = t6("dy"), t6("ys"), t6("dx"), t6("xs")
    ly, y0f, lx, x0f = t6("ly"), t6("y0f"), t6("lx"), t6("x0f")
    hy, hx = t6("hy"), t6("hx")

    V.tensor_tensor(out=dy[:], in0=y2c, in1=y1c, op=OP.subtract)
    V.tensor_scalar(out=ys[:], in0=dy[:], scalar1=cycx[:, 0:1], scalar2=None, op0=OP.mult)
    V.tensor_tensor(out=ys[:], in0=ys[:], in1=y1c, op=OP.add)
    V.tensor_tensor(out=dx[:], in0=x2c, in1=x1c, op=OP.subtract)
    V.tensor_scalar(out=xs[:], in0=dx[:], scalar1=cycx[:, 1:2], scalar2=None, op0=OP.mult)
    V.tensor_tensor(out=xs[:], in0=xs[:], in1=x1c, op=OP.add)
    V.tensor_scalar(out=y0f[:], in0=ys[:], scalar1=-0.5, scalar2=MAGIC, op0=OP.add, op1=OP.add)
    V.tensor_scalar(out=y0f[:], in0=y0f[:], scalar1=-MAGIC, scalar2=None, op0=OP.add)
    V.tensor_tensor(out=ly[:], in0=ys[:], in1=y0f[:], op=OP.subtract)
    V.tensor_scalar(out=x0f[:], in0=xs[:], scalar1=-0.5, scalar2=MAGIC, op0=OP.add, op1=OP.add)
    V.tensor_scalar(out=x0f[:], in0=x0f[:], scalar1=-MAGIC, scalar2=None, op0=OP.add)
    V.tensor_tensor(out=lx[:], in0=xs[:], in1=x0f[:], op=OP.subtract)
    V.tensor_scalar(out=hy[:], in0=ly[:], scalar1=-1.0, scalar2=1.0, op0=OP.mult, op1=OP.add)
    V.tensor_scalar(out=hx[:], in0=lx[:], scalar1=-1.0, scalar2=1.0, op0=OP.mult, op1=OP.add)

    wc = cpool.tile([128, 24], F32, name="wcat")   # free = (ab, g, xc)
    wv = wc[:, :].rearrange("p (ab g x) -> p ab g x", ab=2, x=2)
    V.tensor_tensor(out=wv[:, 0, :, 0], in0=hy[:], in1=hx[:], op=OP.mult)
    V.tensor_tensor(out=wv[:, 0, :, 1], in0=hy[:], in1=lx[:], op=OP.mult)
    V.tensor_tensor(out=wv[:, 1, :, 0], in0=ly[:], in1=hx[:], op=OP.mult)
    V.tensor_tensor(out=wv[:, 1, :, 1], in0=ly[:], in1=lx[:], op=OP.mult)

    # pix = b*H*W + y0*W + x0, clamped; A half then B = A + W
    pixf = t6("pixf")
    V.tensor_scalar(out=pixf[:], in0=y0f[:], scalar1=float(W), scalar2=None, op0=OP.mult)
    V.tensor_tensor(out=pixf[:], in0=pixf[:], in1=x0f[:], op=OP.add)
    V.tensor_tensor(out=pixf[:], in0=pixf[:], in1=bofs[:], op=OP.add)
    V.tensor_scalar(out=pixf[:], in0=pixf[:], scalar1=0.0, scalar2=float(MAX_ROW_A),
                    op0=OP.max, op1=OP.min)
    idx = cpool.tile([128, 12], I32, name="gidx")
    V.tensor_copy(out=idx[:, 0:NG], in_=pixf[:])
    V.tensor_scalar(out=idx[:, NG:12], in0=pixf[:], scalar1=float(W), scalar2=None, op0=OP.add)

    # ---------------- gather: 12 x one-block-per-partition indirect DMAs ----
    G = wpool.tile([128, 12 * 512], F32, name="gather")
    fmv = fm.rearrange("b h w c -> (b h w) c")                    # [32768, 256]
    for j in range(12):
        nc.gpsimd.indirect_dma_start(
            out=G[:, j * 512:(j + 1) * 512],
            out_offset=None,
            in_=fmv,
            in_offset=bass.IndirectOffsetOnAxis(ap=idx[:, j:j + 1], axis=0),
        )

    # ---------------- bilinear combine ----------------
    Gv = G[:, :].rearrange("p (ab g x c) -> p ab g x c", ab=2, g=NG, x=2)
    wb = wc[:, :].rearrange("p (ab g x) -> p ab g x", ab=2, x=2).unsqueeze(4) \
        .to_broadcast([128, 2, NG, 2, C])
    V.tensor_tensor(out=Gv, in0=Gv, in1=wb, op=OP.mult)
    sv2 = wpool.tile([128, NG * 512], F32, name="sv2")
    V.tensor_tensor(out=sv2[:], in0=G[:, 0:3072], in1=G[:, 3072:6144], op=OP.add)
    sv = wpool.tile([128, NG * 256], F32, name="sv")
    s2v = sv2[:, :].rearrange("p (g x c) -> p g x c", g=NG, x=2)
    V.tensor_tensor(out=sv[:, :].rearrange("p (g c) -> p g c", g=NG),
                    in0=s2v[:, :, 0, :], in1=s2v[:, :, 1, :], op=OP.add)

    # ---------------- transpose to feature-major ----------------
    svT = [wpool.tile([128, NG * 128], F32, name=f"svT{h}") for h in range(2)]
    for g in range(NG):
        for h in range(2):
            pt = ppool.tile([128, 128], F32, tag="pt", bufs=4, name="pt")
            nc.tensor.transpose(out=pt[:], in_=sv[:, g * 256 + h * 128: g * 256 + (h + 1) * 128],
                                identity=ident[:])
            nc.scalar.copy(out=svT[h][:, g * 128:(g + 1) * 128], in_=pt[:])

    # ---------------- MLP ----------------
    # psum1 columns j = a*6 + b = roi (a = h in 0..7, b = g in 0..5)
    psum1 = ppool.tile([128, NRS], F32, name="psum1")
    for q in range(16):
        for h in range(2):
            k = q * 2 + h
            rhs = svT[h][:, :].rearrange("p (b a s) -> p a b s", b=6, a=8)[:, :, :, q]
            nc.tensor.matmul(out=psum1[:], lhsT=W1sb[:, k * 128:(k + 1) * 128], rhs=rhs,
                             start=(k == 0), stop=(k == 31))
    l1 = wpool.tile([128, NRS], F32, name="l1")
    nc.scalar.activation(out=l1[:], in_=psum1[:], func=AF.Relu, bias=b1sb[:, 0:1], scale=1.0)

    psum2 = ppool.tile([HID2, NRS], F32, name="psum2")
    nc.tensor.matmul(out=psum2[:], lhsT=W2sb[:, :], rhs=l1[:], start=True, stop=True)
    l2 = wpool.tile([HID2, NRS], F32, name="l2")
    nc.scalar.activation(out=l2[:], in_=psum2[:], func=AF.Relu, bias=b2sb[:, 0:1], scale=1.0)

    psum3 = ppool.tile([NRS, NUM_CLASSES], F32, name="psum3")
    nc.tensor.matmul(out=psum3[:], lhsT=l2[:], rhs=W3sb[:], start=True, stop=True)

    # ---------------- softmax (rows 0..43 only) ----------------
    logits = wpool.tile([NROI, NUM_CLASSES], F32, name="logits")
    V.tensor_tensor(out=logits[:], in0=psum3[0:NROI, :], in1=b3sb[:], op=OP.add)
    mxn = wpool.tile([NROI, 1], F32, name="mxn")
    V.tensor_reduce(out=mxn[:], in_=logits[:], axis=AX_X, op=OP.max, negate=True)
    ex = wpool.tile([NROI, NUM_CLASSES], F32, name="ex")
    nc.scalar.activation(out=ex[:], in_=logits[:], func=AF.Exp, bias=mxn[:, 0:1], scale=1.0)
    ssum = wpool.tile([NROI, 1], F32, name="ssum")
    V.tensor_reduce(out=ssum[:], in_=ex[:], axis=AX_X, op=OP.add)
    rinv = wpool.tile([NROI, 1], F32, name="rinv")
    V.reciprocal(rinv[:], ssum[:])
    probs = wpool.tile([NROI, NUM_CLASSES], F32, name="probs")
    V.tensor_scalar(out=probs[:], in0=ex[:], scalar1=rinv[:, 0:1], scalar2=None, op0=OP.mult)

    nc.sync.dma_start(out.rearrange("b p c -> (b p) c"), probs[:])


def build_module():
    nc = bacc.Bacc(get_trn_type() or "TRN2", target_bir_lowering=False, debug=False)
    fm = nc.dram_tensor("feature_map", [B_LOC, H, W, C], F32, kind="ExternalInput")
    prop = nc.dram_tensor("proposals", [B_LOC, P, 4], F32, kind="ExternalInput")
    W1 = nc.dram_tensor("W1", [4096, HID1], F32, kind="ExternalInput")
    b1 = nc.dram_tensor("b1", [HID1], F32, kind="ExternalInput")
    W2 = nc.dram_tensor("W2", [HID1, HID2], F32, kind="ExternalInput")
    b2 = nc.dram_tensor("b2", [HID2], F32, kind="ExternalInput")
    W3 = nc.dram_tensor("W3", [HID2, NUM_CLASSES], F32, kind="ExternalInput")
    b3 = nc.dram_tensor("b3", [NUM_CLASSES], F32, kind="ExternalInput")
    out = nc.dram_tensor("out", [B_LOC, P, NUM_CLASSES], F32, kind="ExternalOutput")

    ident_np, cycx_np, bofs_np = _static_consts()
    ident_c = nc.inline_tensor(ident_np, name="c_ident")
    cycx_c = nc.inline_tensor(cycx_np, name="c_cycx")
    bofs_c = nc.inline_tensor(bofs_np, name="c_bofs")

    with tile.TileContext(nc) as tc:
        emit_kernel(nc, tc, fm[:], prop[:], W1[:], b1[:], W2[:], b2[:], W3[:], b3[:],
                    out[:], (ident_c[:], cycx_c[:], bofs_c[:]))
    nc.compile()
    return nc


_NC_CACHE = None


def _get_module():
    global _NC_CACHE
    if _NC_CACHE is None:
        _NC_CACHE = build_module()
    return _NC_CACHE


def _shard_inputs(inputs):
    f = {k: np.ascontiguousarray(np.asarray(v, dtype=np.float32)) for k, v in inputs.items()}
    in_maps = []
    for c in range(N_CORES):
        sl = slice(B_LOC * c, B_LOC * (c + 1))
        in_maps.append({
            "feature_map": f["feature_map"][sl],
            "proposals": f["proposals"][sl],
            "W1": f["W1"], "b1": f["b1"],
            "W2": f["W2"], "b2": f["b2"],
            "W3": f["W3"], "b3": f["b3"],
        })
    return in_maps


def run(inputs, trace=False):
    """Run on all 8 cores; returns (output [16,22,10], BassKernelResults)."""
    nc = _get_module()
    res = run_bass_kernel_spmd(nc, _shard_inputs(inputs), core_ids=list(range(N_CORES)),
                               trace=trace)
    out = np.concatenate([r["out"] for r in res.results], axis=0)
    return out, res


def kernel(**inputs) -> np.ndarray:
    out, _ = run(inputs, trace=False)
    return out


# revision 15
# speedup vs baseline: 1.2310x; 1.2310x over previous
"""ROI-Align + MLP classification head (nms_detection) on 8 Trainium2 cores.

Strategy: data-parallel over batch (2 images per core). Per core, the kernel
computes bilinear sample coordinates from the proposals on-device, gathers
only the needed feature-map pixel pairs with indirect DMAs (~3 MB instead
of streaming the full 32 MB shard), does the bilinear combine on the vector
engine, transposes sample-major -> feature-major on the PE, and runs the
3-layer MLP + softmax.

Layouts (per core): 44 rois x 16 bin-centers = 704 samples.
  roi slot (h, g): roi = h*6 + g, h in 0..7, g in 0..5 (48 slots, 4 dup/garbage)
  sample partition p = h*16 + q (q = iy*4+ix), sample group = g.
  gather block j = ab*6 + g (ab = y-corner row 0/1), one indirect DMA each:
    G[p, j*512 :+512] = fm row pair (y0+ab, x0..x0+1) channels (512 floats).
"""

import numpy as np

import concourse.bacc as bacc
import concourse.bass as bass
import concourse.mybir as mybir
import concourse.tile as tile
from concourse._compat import get_trn_type
from concourse.bass_utils import run_bass_kernel_spmd

# Problem shape (hardcoded per contract)
B, P, H, W, C = 16, 22, 128, 128, 256
NUM_CLASSES = 10
N_CORES = 8
B_LOC = B // N_CORES        # 2 images per core
NROI = B_LOC * P            # 44 rois per core
NRS = 48                    # roi slots (8 partition-blocks x 6 groups)
NG = 6                      # sample groups of 128
HID1, HID2 = 128, 64
F32 = mybir.dt.float32
BF16 = mybir.dt.bfloat16
I32 = mybir.dt.int32
AX_X = mybir.AxisListType.X
OP = mybir.AluOpType
AF = mybir.ActivationFunctionType

NPIX = B_LOC * H * W            # 32768 flat pixel rows per core
MAX_ROW_A = NPIX - 130          # room for +1 col pair and +W row
MAGIC = 12582912.0              # 1.5 * 2^23 fp32 round-to-int magic


def _static_consts():
    import ml_dtypes
    ident = np.eye(128).astype(ml_dtypes.bfloat16)
    p = np.arange(128)
    q = p % 16
    cy = ((q // 4).astype(np.float32) + 0.5) / 4.0
    cx = ((q % 4).astype(np.float32) + 0.5) / 4.0
    cycx = np.stack([cy, cx], axis=1).astype(np.float32)          # [128, 2]
    # per-sample batch offset: bofs[p, g] for roi = (p//16)*6 + g
    h = np.arange(128)[:, None] // 16
    g = np.arange(NG)[None, :]
    roi = h * 6 + g                                               # [128, 6]
    bofs = np.where(roi >= P, float(H * W), 0.0).astype(np.float32)
    return ident, cycx, bofs


def emit_kernel(nc, tc, fm, prop, W1, b1, W2, b2, W3, b3, out, consts):
    """Emit the per-core tile kernel. All args are bass.APs."""
    with (
        tc.tile_pool(name="const", bufs=1) as cpool,
        tc.tile_pool(name="work", bufs=1) as wpool,
        tc.tile_pool(name="psum", bufs=1, space="PSUM") as ppool,
    ):
        _emit_body(nc, tc, fm, prop, W1, b1, W2, b2, W3, b3, out, consts,
                   cpool, wpool, ppool)


def _coords_expand(nc, V, prop, Ct, order):
    """Broadcast-replicate proposals into per-partition layout.

    order='gk': Ct[p, g*4+k] = prop[roi(p//16, g), k]          ([128, 24])
    order='gh': Ct[p, g*32 + h*4 + k] = prop[roi(h, g), k]     ([128, 192])
    roi(h, g) = h*6 + g, with the h=7 block read from roi 38 (clamped) and
    fixed up so (h=7, g=0,1) hold rois 42, 43.
    """
    pv = prop.rearrange("b p k -> (b p k)")                        # [176]
    for h in range(8):
        start = min(h * 6, 38) * 4
        src = pv[start:start + 24].unsqueeze(0)                    # [1, 24]
        if order == "gk":
            dst = Ct[h * 16:(h + 1) * 16, 0:24].rearrange("p (g k) -> p g k", g=6)
            nc.sync.dma_start(dst, src.rearrange("o (g k) -> o g k", g=6)
                              .to_broadcast([16, 6, 4]))
        else:
            dst = Ct[:, :].rearrange("p (g h k) -> p g h k", g=6, h=8)[:, :, h, :]
            nc.sync.dma_start(dst, src.rearrange("o (g k) -> o g k", g=6)
                              .to_broadcast([128, 6, 4]))
    # h=7 fixup: slots (g=0,1) must hold rois 42,43 (read rois 38..43)
    if order == "gk":
        nc.sync.dma_start(Ct[112:128, 0:8], Ct[112:128, 16:24])
    else:
        vw = Ct[:, :].rearrange("p (g h k) -> p g h k", g=6, h=8)
        V.tensor_copy(out=vw[:, 0:2, 7, :], in_=vw[:, 4:6, 7, :])


def _emit_body(nc, tc, fm, prop, W1, b1, W2, b2, W3, b3, out, consts,
               cpool, wpool, ppool):
    ident_c, cycx_c, bofs_c = consts
    V = nc.vector

    # ---------------- small consts needed by the index chain ----------------
    ident = cpool.tile([128, 128], BF16, name="ident")
    nc.sync.dma_start(ident[:], ident_c)
    cycx = cpool.tile([128, 2], F32, name="cycx")
    nc.sync.dma_start(cycx[:], cycx_c)
    bofs = cpool.tile([128, NG], F32, name="bofs")
    nc.sync.dma_start(bofs[:], bofs_c)

    # ============ coords + weights + indices (per-sample layout) ============
    Ct = cpool.tile([128, NG * 4], F32, name="coords")
    _coords_expand(nc, V, prop, Ct, "gk")
    cgv = Ct[:, :].rearrange("p (g k) -> p g k", g=NG)
    y1c, x1c, y2c, x2c = (cgv[:, :, k] for k in range(4))

    def t6(name):
        return wpool.tile([128, NG], F32, name=name)

    dy, ys, dx, xs = t6("dy"), t6("ys"), t6("dx"), t6("xs")
    ly, y0f, lx, x0f = t6("ly"), t6("y0f"), t6("lx"), t6("x0f")
    hy, hx = t6("hy"), t6("hx")

    V.tensor_tensor(out=dy[:], in0=y2c, in1=y1c, op=OP.subtract)
    V.tensor_scalar(out=ys[:], in0=dy[:], scalar1=cycx[:, 0:1], scalar2=None, op0=OP.mult)
    V.tensor_tensor(out=ys[:], in0=ys[:], in1=y1c, op=OP.add)
    V.tensor_tensor(out=dx[:], in0=x2c, in1=x1c, op=OP.subtract)
    V.tensor_scalar(out=xs[:], in0=dx[:], scalar1=cycx[:, 1:2], scalar2=None, op0=OP.mult)
    V.tensor_tensor(out=xs[:], in0=xs[:], in1=x1c, op=OP.add)
    V.tensor_scalar(out=y0f[:], in0=ys[:], scalar1=-0.5, scalar2=MAGIC, op0=OP.add, op1=OP.add)
    V.tensor_scalar(out=y0f[:], in0=y0f[:], scalar1=-MAGIC, scalar2=None, op0=OP.add)
    V.tensor_scalar(out=x0f[:], in0=xs[:], scalar1=-0.5, scalar2=MAGIC, op0=OP.add, op1=OP.add)
    V.tensor_scalar(out=x0f[:], in0=x0f[:], scalar1=-MAGIC, scalar2=None, op0=OP.add)

    # gather indices first (critical path): pix = b*H*W + y0*W + x0
    pixf = t6("pixf")
    V.tensor_scalar(out=pixf[:], in0=y0f[:], scalar1=float(W), scalar2=None, op0=OP.mult)
    V.tensor_tensor(out=pixf[:], in0=pixf[:], in1=x0f[:], op=OP.add)
    V.tensor_tensor(out=pixf[:], in0=pixf[:], in1=bofs[:], op=OP.add)
    V.tensor_scalar(out=pixf[:], in0=pixf[:], scalar1=0.0, scalar2=float(MAX_ROW_A),
                    op0=OP.max, op1=OP.min)
    idx = cpool.tile([128, 12], I32, name="gidx")
    V.tensor_copy(out=idx[:, 0:NG], in_=pixf[:])
    V.tensor_scalar(out=idx[:, NG:12], in0=pixf[:], scalar1=float(W), scalar2=None, op0=OP.add)

    # ---------------- gather: 12 indirect DMAs, chunk-1 groups first --------
    G = wpool.tile([128, 12 * 512], F32, name="gather")
    fmv = fm.rearrange("b h w c -> (b h w) c")                    # [32768, 256]
    for j in (0, 1, 2, 6, 7, 8, 3, 4, 5, 9, 10, 11):
        nc.gpsimd.indirect_dma_start(
            out=G[:, j * 512:(j + 1) * 512],
            out_offset=None,
            in_=fmv,
            in_offset=bass.IndirectOffsetOnAxis(ap=idx[:, j:j + 1], axis=0),
        )

    # weights for the bilinear combine (off the gather critical path)
    V.tensor_tensor(out=ly[:], in0=ys[:], in1=y0f[:], op=OP.subtract)
    V.tensor_tensor(out=lx[:], in0=xs[:], in1=x0f[:], op=OP.subtract)
    V.tensor_scalar(out=hy[:], in0=ly[:], scalar1=-1.0, scalar2=1.0, op0=OP.mult, op1=OP.add)
    V.tensor_scalar(out=hx[:], in0=lx[:], scalar1=-1.0, scalar2=1.0, op0=OP.mult, op1=OP.add)
    wc = cpool.tile([128, 24], F32, name="wcat")   # free = (ab, g, xc)
    wv = wc[:, :].rearrange("p (ab g x) -> p ab g x", ab=2, x=2)
    V.tensor_tensor(out=wv[:, 0, :, 0], in0=hy[:], in1=hx[:], op=OP.mult)
    V.tensor_tensor(out=wv[:, 0, :, 1], in0=hy[:], in1=lx[:], op=OP.mult)
    V.tensor_tensor(out=wv[:, 1, :, 0], in0=ly[:], in1=hx[:], op=OP.mult)
    V.tensor_tensor(out=wv[:, 1, :, 1], in0=ly[:], in1=lx[:], op=OP.mult)

    # ---------------- weight/bias loads (overlap the gather) ----------------
    W1f = cpool.tile([128, 4096], F32, name="W1f")
    nc.sync.dma_start(W1f[:, :].rearrange("p (k h) -> p k h", k=32),
                      W1.rearrange("(k p) h -> p k h", p=128))
    W1sb = cpool.tile([128, 4096], BF16, name="W1sb")
    nc.scalar.copy(out=W1sb[:], in_=W1f[:])
    W2f = cpool.tile([128, HID2], F32, name="W2f")
    nc.sync.dma_start(W2f[:], W2)
    W2sb = cpool.tile([128, HID2], BF16, name="W2sb")
    nc.scalar.copy(out=W2sb[:], in_=W2f[:])
    W3f = cpool.tile([HID2, NUM_CLASSES], F32, name="W3f")
    nc.sync.dma_start(W3f[:], W3)
    W3sb = cpool.tile([HID2, NUM_CLASSES], BF16, name="W3sb")
    nc.scalar.copy(out=W3sb[:], in_=W3f[:])
    b1sb = cpool.tile([128, 1], F32, name="b1sb")
    nc.sync.dma_start(b1sb[:], b1.rearrange("(p o) -> p o", o=1))
    b2sb = cpool.tile([HID2, 1], F32, name="b2sb")
    nc.sync.dma_start(b2sb[:], b2.rearrange("(p o) -> p o", o=1))
    b3sb = cpool.tile([NROI, NUM_CLASSES], F32, name="b3sb")
    nc.sync.dma_start(b3sb[:], b3.unsqueeze(0).to_broadcast([NROI, NUM_CLASSES]))

    # ---------------- bilinear combine + transpose, 2 group-chunks ----------
    Gv = G[:, :].rearrange("p (ab g x c) -> p ab g x c", ab=2, g=NG, x=2)
    wb = wc[:, :].rearrange("p (ab g x) -> p ab g x", ab=2, x=2).unsqueeze(4) \
        .to_broadcast([128, 2, NG, 2, C])
    sv2 = wpool.tile([128, NG * 512], F32, name="sv2")
    sv = wpool.tile([128, NG * 256], BF16, name="sv")
    s2v = sv2[:, :].rearrange("p (g x c) -> p g x c", g=NG, x=2)
    svv = sv[:, :].rearrange("p (g c) -> p g c", g=NG)
    svT = [wpool.tile([128, NG * 128], BF16, name=f"svT{h}") for h in range(2)]
    half = NG // 2
    for cidx in range(2):
        gs = slice(cidx * half, (cidx + 1) * half)
        V.tensor_tensor(out=Gv[:, :, gs, :, :], in0=Gv[:, :, gs, :, :],
                        in1=wb[:, :, gs, :, :], op=OP.mult)
        V.tensor_tensor(out=sv2[:, cidx * 1536:(cidx + 1) * 1536],
                        in0=G[:, cidx * 1536:cidx * 1536 + 1536],
                        in1=G[:, 3072 + cidx * 1536:3072 + cidx * 1536 + 1536],
                        op=OP.add)
        V.tensor_tensor(out=svv[:, gs, :], in0=s2v[:, gs, 0, :], in1=s2v[:, gs, 1, :],
                        op=OP.add)
        for g in range(cidx * half, (cidx + 1) * half):
            for h in range(2):
                pt = ppool.tile([128, 128], BF16, tag="pt", bufs=4, name="pt")
                nc.tensor.transpose(out=pt[:],
                                    in_=sv[:, g * 256 + h * 128: g * 256 + (h + 1) * 128],
                                    identity=ident[:])
                nc.scalar.copy(out=svT[h][:, g * 128:(g + 1) * 128], in_=pt[:])

    # ---------------- MLP ----------------
    # psum1 columns j = a*6 + b = roi (a = h in 0..7, b = g in 0..5)
    psum1 = ppool.tile([128, NRS], F32, name="psum1")
    for q in range(16):
        for h in range(2):
            k = q * 2 + h
            rhs = svT[h][:, :].rearrange("p (b a s) -> p a b s", b=6, a=8)[:, :, :, q]
            nc.tensor.matmul(out=psum1[:], lhsT=W1sb[:, k * 128:(k + 1) * 128], rhs=rhs,
                             start=(k == 0), stop=(k == 31))
    l1 = wpool.tile([128, NRS], BF16, name="l1")
    nc.scalar.activation(out=l1[:], in_=psum1[:], func=AF.Relu, bias=b1sb[:, 0:1], scale=1.0)

    psum2 = ppool.tile([HID2, NRS], F32, name="psum2")
    nc.tensor.matmul(out=psum2[:], lhsT=W2sb[:, :], rhs=l1[:], start=True, stop=True)
    l2 = wpool.tile([HID2, NRS], BF16, name="l2")
    nc.scalar.activation(out=l2[:], in_=psum2[:], func=AF.Relu, bias=b2sb[:, 0:1], scale=1.0)

    psum3 = ppool.tile([NRS, NUM_CLASSES], F32, name="psum3")
    nc.tensor.matmul(out=psum3[:], lhsT=l2[:], rhs=W3sb[:], start=True, stop=True)

    # ---------------- softmax (rows 0..43 only) ----------------
    logits = wpool.tile([NROI, NUM_CLASSES], F32, name="logits")
    V.tensor_tensor(out=logits[:], in0=psum3[0:NROI, :], in1=b3sb[:], op=OP.add)
    mxn = wpool.tile([NROI, 1], F32, name="mxn")
    V.tensor_reduce(out=mxn[:], in_=logits[:], axis=AX_X, op=OP.max, negate=True)
    ex = wpool.tile([NROI, NUM_CLASSES], F32, name="ex")
    nc.scalar.activation(out=ex[:], in_=logits[:], func=AF.Exp, bias=mxn[:, 0:1], scale=1.0)
    ssum = wpool.tile([NROI, 1], F32, name="ssum")
    V.tensor_reduce(out=ssum[:], in_=ex[:], axis=AX_X, op=OP.add)
    rinv = wpool.tile([NROI, 1], F32, name="rinv")
    V.reciprocal(rinv[:], ssum[:])
    probs = wpool.tile([NROI, NUM_CLASSES], F32, name="probs")
    V.tensor_scalar(out=probs[:], in0=ex[:], scalar1=rinv[:, 0:1], scalar2=None, op0=OP.mult)

    nc.sync.dma_start(out.rearrange("b p c -> (b p) c"), probs[:])


def build_module():
    nc = bacc.Bacc(get_trn_type() or "TRN2", target_bir_lowering=False, debug=False)
    fm = nc.dram_tensor("feature_map", [B_LOC, H, W, C], F32, kind="ExternalInput")
    prop = nc.dram_tensor("proposals", [B_LOC, P, 4], F32, kind="ExternalInput")
    W1 = nc.dram_tensor("W1", [4096, HID1], F32, kind="ExternalInput")
    b1 = nc.dram_tensor("b1", [HID1], F32, kind="ExternalInput")
    W2 = nc.dram_tensor("W2", [HID1, HID2], F32, kind="ExternalInput")
    b2 = nc.dram_tensor("b2", [HID2], F32, kind="ExternalInput")
    W3 = nc.dram_tensor("W3", [HID2, NUM_CLASSES], F32, kind="ExternalInput")
    b3 = nc.dram_tensor("b3", [NUM_CLASSES], F32, kind="ExternalInput")
    out = nc.dram_tensor("out", [B_LOC, P, NUM_CLASSES], F32, kind="ExternalOutput")

    ident_np, cycx_np, bofs_np = _static_consts()
    ident_c = nc.inline_tensor(ident_np, name="c_ident")
    cycx_c = nc.inline_tensor(cycx_np, name="c_cycx")
    bofs_c = nc.inline_tensor(bofs_np, name="c_bofs")

    with tile.TileContext(nc) as tc:
        emit_kernel(nc, tc, fm[:], prop[:], W1[:], b1[:], W2[:], b2[:], W3[:], b3[:],
                    out[:], (ident_c[:], cycx_c[:], bofs_c[:]))
    nc.compile()
    return nc


_NC_CACHE = None


def _get_module():
    global _NC_CACHE
    if _NC_CACHE is None:
        _NC_CACHE = build_module()
    return _NC_CACHE


def _shard_inputs(inputs):
    f = {k: np.ascontiguousarray(np.asarray(v, dtype=np.float32)) for k, v in inputs.items()}
    in_maps = []
    for c in range(N_CORES):
        sl = slice(B_LOC * c, B_LOC * (c + 1))
        in_maps.append({
            "feature_map": f["feature_map"][sl],
            "proposals": f["proposals"][sl],
            "W1": f["W1"], "b1": f["b1"],
            "W2": f["W2"], "b2": f["b2"],
            "W3": f["W3"], "b3": f["b3"],
        })
    return in_maps


def run(inputs, trace=False):
    """Run on all 8 cores; returns (output [16,22,10], BassKernelResults)."""
    nc = _get_module()
    res = run_bass_kernel_spmd(nc, _shard_inputs(inputs), core_ids=list(range(N_CORES)),
                               trace=trace)
    out = np.concatenate([r["out"] for r in res.results], axis=0)
    return out, res


def kernel(**inputs) -> np.ndarray:
    out, _ = run(inputs, trace=False)
    return out


# revision 16
# speedup vs baseline: 1.3872x; 1.1269x over previous
"""ROI-Align + MLP classification head (nms_detection) on 8 Trainium2 cores.

Strategy: data-parallel over batch (2 images per core). Per core, the kernel
computes bilinear sample coordinates from the proposals on-device, gathers
only the needed feature-map pixel pairs with indirect DMAs (~3 MB instead of
streaming the full 32 MB shard, cast to fp16 in-flight), does the bilinear
combine on the vector engine in fp16, transposes sample-major ->
feature-major on the PE, and runs the 3-layer MLP (fp16 in / fp32 psum) +
fp32 softmax.

Layouts (per core): 44 rois x 16 bin-centers = 704 samples.
  roi slot (h, g): roi = h*6 + g, h in 0..7, g in 0..5 (48 slots, 4 dup/garbage)
  sample partition p = h*16 + q (q = iy*4+ix), sample group = g.
  gather block j = ab*6 + g (ab = y-corner row 0/1), one indirect DMA each:
    G[p, j*512 :+512] = fm row pair (y0+ab, x0..x0+1) channels (512 floats).
"""

import numpy as np

import concourse.bacc as bacc
import concourse.bass as bass
import concourse.mybir as mybir
import concourse.tile as tile
from concourse._compat import get_trn_type
from concourse.bass_utils import run_bass_kernel_spmd

# Problem shape (hardcoded per contract)
B, P, H, W, C = 16, 22, 128, 128, 256
NUM_CLASSES = 10
N_CORES = 8
B_LOC = B // N_CORES        # 2 images per core
NROI = B_LOC * P            # 44 rois per core
NRS = 48                    # roi slots (8 partition-blocks x 6 groups)
NG = 6                      # sample groups of 128
HID1, HID2 = 128, 64
F32 = mybir.dt.float32
F16 = mybir.dt.float16
I32 = mybir.dt.int32
AX_X = mybir.AxisListType.X
OP = mybir.AluOpType
AF = mybir.ActivationFunctionType

NPIX = B_LOC * H * W            # 32768 flat pixel rows per core
MAX_ROW_A = NPIX - 130          # room for +1 col pair and +W row
MAGIC = 12582912.0              # 1.5 * 2^23 fp32 round-to-int magic


def _static_consts():
    ident = np.eye(128).astype(np.float16)
    p = np.arange(128)
    q = p % 16
    cy = ((q // 4).astype(np.float32) + 0.5) / 4.0
    cx = ((q % 4).astype(np.float32) + 0.5) / 4.0
    # per-sample batch offset: bofs[p, g] for roi = (p//16)*6 + g
    h = np.arange(128)[:, None] // 16
    g = np.arange(NG)[None, :]
    roi = h * 6 + g                                               # [128, 6]
    bofs = np.where(roi >= P, float(H * W), 0.0).astype(np.float32)
    cb32 = np.concatenate([cy[:, None], cx[:, None], bofs], axis=1).astype(np.float32)
    cidx = np.minimum(np.arange(128) // 16 * 6, 38).astype(np.int32)[:, None]  # [128,1]
    return ident, cb32, cidx


def emit_kernel(nc, tc, fm, prop, W1, b1, W2, b2, W3, b3, out, consts):
    """Emit the per-core tile kernel. All args are bass.APs."""
    with (
        tc.tile_pool(name="const", bufs=1) as cpool,
        tc.tile_pool(name="work", bufs=1) as wpool,
        tc.tile_pool(name="psum", bufs=1, space="PSUM") as ppool,
    ):
        _emit_body(nc, tc, fm, prop, W1, b1, W2, b2, W3, b3, out, consts,
                   cpool, wpool, ppool)


def _emit_body(nc, tc, fm, prop, W1, b1, W2, b2, W3, b3, out, consts,
               cpool, wpool, ppool):
    ident_c, cb32_c, cidx_c = consts
    V = nc.vector

    # ---------------- bundled consts (3 small DMAs) ----------------
    ident = cpool.tile([128, 128], F16, name="ident")
    nc.sync.dma_start(ident[:], ident_c)
    cb32 = cpool.tile([128, 8], F32, name="cb32")
    nc.sync.dma_start(cb32[:], cb32_c)
    cidx = cpool.tile([128, 1], I32, name="cidx")
    nc.sync.dma_start(cidx[:], cidx_c)
    cy_ap, cx_ap, bofs = cb32[:, 0:1], cb32[:, 1:2], cb32[:, 2:8]

    # ---------------- coords expansion: one POOL gather + fixup ----------
    # Ct[p, g*4+k] = proposals[roi(p//16, g), k]; h=7 block reads rois 38..43
    Ct = cpool.tile([128, NG * 4], F32, name="coords")
    pv = prop.rearrange("b p k -> (b p) k")                       # [44, 4]
    nc.gpsimd.indirect_dma_start(
        out=Ct[:], out_offset=None, in_=pv,
        in_offset=bass.IndirectOffsetOnAxis(ap=cidx[:, :], axis=0))
    # h=7 fixup: slots (g=0,1) must hold rois 42,43
    nc.sync.dma_start(Ct[112:128, 0:8], Ct[112:128, 16:24])

    cgv = Ct[:, :].rearrange("p (g k) -> p g k", g=NG)
    y1c, x1c, y2c, x2c = (cgv[:, :, k] for k in range(4))

    # ---------------- index chain (critical path to the gathers) --------
    def t6(name):
        return wpool.tile([128, NG], F32, name=name)

    dy, ys, dx, xs = t6("dy"), t6("ys"), t6("dx"), t6("xs")
    ly, y0f, lx, x0f = t6("ly"), t6("y0f"), t6("lx"), t6("x0f")
    hy, hx, pixf = t6("hy"), t6("hx"), t6("pixf")

    V.tensor_tensor(out=dy[:], in0=y2c, in1=y1c, op=OP.subtract)
    V.tensor_scalar(out=ys[:], in0=dy[:], scalar1=cy_ap, scalar2=None, op0=OP.mult)
    V.tensor_tensor(out=ys[:], in0=ys[:], in1=y1c, op=OP.add)
    V.tensor_tensor(out=dx[:], in0=x2c, in1=x1c, op=OP.subtract)
    V.tensor_scalar(out=xs[:], in0=dx[:], scalar1=cx_ap, scalar2=None, op0=OP.mult)
    V.tensor_tensor(out=xs[:], in0=xs[:], in1=x1c, op=OP.add)
    # y0 = round(ys - 0.5) via fp32 magic; consistent-pair bilinear stays exact
    V.tensor_scalar(out=y0f[:], in0=ys[:], scalar1=-0.5, scalar2=MAGIC, op0=OP.add, op1=OP.add)
    V.tensor_scalar(out=y0f[:], in0=y0f[:], scalar1=-MAGIC, scalar2=None, op0=OP.add)
    V.tensor_scalar(out=x0f[:], in0=xs[:], scalar1=-0.5, scalar2=MAGIC, op0=OP.add, op1=OP.add)
    V.tensor_scalar(out=x0f[:], in0=x0f[:], scalar1=-MAGIC, scalar2=None, op0=OP.add)
    # pix = b*H*W + y0*W + x0, clamped
    V.tensor_scalar(out=pixf[:], in0=y0f[:], scalar1=float(W), scalar2=None, op0=OP.mult)
    V.tensor_tensor(out=pixf[:], in0=pixf[:], in1=x0f[:], op=OP.add)
    V.tensor_tensor(out=pixf[:], in0=pixf[:], in1=bofs, op=OP.add)
    V.tensor_scalar(out=pixf[:], in0=pixf[:], scalar1=0.0, scalar2=float(MAX_ROW_A),
                    op0=OP.max, op1=OP.min)
    idx = cpool.tile([128, 12], I32, name="gidx")
    V.tensor_copy(out=idx[:, 0:NG], in_=pixf[:])
    V.tensor_scalar(out=idx[:, NG:12], in0=pixf[:], scalar1=float(W), scalar2=None, op0=OP.add)

    # ---------------- gather: 12 indirect DMAs (fp16 cast in-flight) ------
    G = wpool.tile([128, 12 * 512], F16, name="gather")
    fmv = fm.rearrange("b h w c -> (b h w) c")                    # [32768, 256]
    half = NG // 2
    CHUNK_J = [(0, 1, 2, 6, 7, 8), (3, 4, 5, 9, 10, 11)]
    for js in CHUNK_J:
        for j in js:
            nc.gpsimd.indirect_dma_start(
                out=G[:, j * 512:(j + 1) * 512],
                out_offset=None,
                in_=fmv,
                in_offset=bass.IndirectOffsetOnAxis(ap=idx[:, j:j + 1], axis=0),
            )

    # bilinear corner weights (off the gather critical path), fp16
    V.tensor_tensor(out=ly[:], in0=ys[:], in1=y0f[:], op=OP.subtract)
    V.tensor_tensor(out=lx[:], in0=xs[:], in1=x0f[:], op=OP.subtract)
    V.tensor_scalar(out=hy[:], in0=ly[:], scalar1=-1.0, scalar2=1.0, op0=OP.mult, op1=OP.add)
    V.tensor_scalar(out=hx[:], in0=lx[:], scalar1=-1.0, scalar2=1.0, op0=OP.mult, op1=OP.add)
    wc = cpool.tile([128, 24], F16, name="wcat")   # free = (ab, g, xc)
    wv = wc[:, :].rearrange("p (ab g x) -> p ab g x", ab=2, x=2)
    V.tensor_tensor(out=wv[:, 0, :, 0], in0=hy[:], in1=hx[:], op=OP.mult)
    V.tensor_tensor(out=wv[:, 0, :, 1], in0=hy[:], in1=lx[:], op=OP.mult)
    V.tensor_tensor(out=wv[:, 1, :, 0], in0=ly[:], in1=hx[:], op=OP.mult)
    V.tensor_tensor(out=wv[:, 1, :, 1], in0=ly[:], in1=lx[:], op=OP.mult)

    # ---------------- weight/bias loads (overlap the gather) --------------
    W1f = cpool.tile([128, 4096], F32, name="W1f")
    nc.sync.dma_start(W1f[:, :].rearrange("p (k h) -> p k h", k=32),
                      W1.rearrange("(k p) h -> p k h", p=128))
    W1sb = cpool.tile([128, 4096], F16, name="W1sb")
    nc.scalar.copy(out=W1sb[:], in_=W1f[:])
    W2f = cpool.tile([128, HID2], F32, name="W2f")
    nc.sync.dma_start(W2f[:], W2)
    W2sb = cpool.tile([128, HID2], F16, name="W2sb")
    nc.scalar.copy(out=W2sb[:], in_=W2f[:])
    W3f = cpool.tile([HID2, NUM_CLASSES], F32, name="W3f")
    nc.sync.dma_start(W3f[:], W3)
    W3sb = cpool.tile([HID2, NUM_CLASSES], F16, name="W3sb")
    nc.scalar.copy(out=W3sb[:], in_=W3f[:])
    b1sb = cpool.tile([128, 1], F32, name="b1sb")
    nc.sync.dma_start(b1sb[:], b1.rearrange("(p o) -> p o", o=1))
    b2sb = cpool.tile([HID2, 1], F32, name="b2sb")
    nc.sync.dma_start(b2sb[:], b2.rearrange("(p o) -> p o", o=1))
    b3sb = cpool.tile([NROI, NUM_CLASSES], F32, name="b3sb")
    nc.sync.dma_start(b3sb[:], b3.unsqueeze(0).to_broadcast([NROI, NUM_CLASSES]))

    # ---------------- bilinear combine + transpose, 2 group-chunks --------
    Gv = G[:, :].rearrange("p (ab g x c) -> p ab g x c", ab=2, g=NG, x=2)
    wb = wc[:, :].rearrange("p (ab g x) -> p ab g x", ab=2, x=2).unsqueeze(4) \
        .to_broadcast([128, 2, NG, 2, C])
    sv2 = wpool.tile([128, NG * 512], F16, name="sv2")
    sv = wpool.tile([128, NG * 256], F16, name="sv")
    s2v = sv2[:, :].rearrange("p (g x c) -> p g x c", g=NG, x=2)
    svv = sv[:, :].rearrange("p (g c) -> p g c", g=NG)
    svT = [wpool.tile([128, NG * 128], F16, name=f"svT{h}") for h in range(2)]
    for cix in range(2):
        gs = slice(cix * half, (cix + 1) * half)
        V.tensor_tensor(out=Gv[:, :, gs, :, :], in0=Gv[:, :, gs, :, :],
                        in1=wb[:, :, gs, :, :], op=OP.mult)
        V.tensor_tensor(out=sv2[:, cix * 1536:(cix + 1) * 1536],
                        in0=G[:, cix * 1536:cix * 1536 + 1536],
                        in1=G[:, 3072 + cix * 1536:3072 + cix * 1536 + 1536],
                        op=OP.add)
        V.tensor_tensor(out=svv[:, gs, :], in0=s2v[:, gs, 0, :], in1=s2v[:, gs, 1, :],
                        op=OP.add)
        for h in range(2):
            for g in range(cix * half, (cix + 1) * half):
                pt = ppool.tile([128, 128], F16, tag="pt", bufs=4, name="pt")
                nc.tensor.transpose(out=pt[:],
                                    in_=sv[:, g * 256 + h * 128: g * 256 + (h + 1) * 128],
                                    identity=ident[:])
                nc.scalar.copy(out=svT[h][:, g * 128:(g + 1) * 128], in_=pt[:])

    # ---------------- MLP ----------------
    # psum1 columns j = a*6 + b = roi (a = h in 0..7, b = g in 0..5)
    psum1 = ppool.tile([128, NRS], F32, name="psum1")
    for h in range(2):
        for q in range(16):
            k = q * 2 + h
            rhs = svT[h][:, :].rearrange("p (b a s) -> p a b s", b=6, a=8)[:, :, :, q]
            nc.tensor.matmul(out=psum1[:], lhsT=W1sb[:, k * 128:(k + 1) * 128], rhs=rhs,
                             start=(h == 0 and q == 0), stop=(h == 1 and q == 15))
    l1 = wpool.tile([128, NRS], F16, name="l1")
    nc.scalar.activation(out=l1[:], in_=psum1[:], func=AF.Relu, bias=b1sb[:, 0:1], scale=1.0)

    psum2 = ppool.tile([HID2, NRS], F32, name="psum2")
    nc.tensor.matmul(out=psum2[:], lhsT=W2sb[:, :], rhs=l1[:], start=True, stop=True)
    l2 = wpool.tile([HID2, NRS], F16, name="l2")
    nc.scalar.activation(out=l2[:], in_=psum2[:], func=AF.Relu, bias=b2sb[:, 0:1], scale=1.0)

    psum3 = ppool.tile([NRS, NUM_CLASSES], F32, name="psum3")
    nc.tensor.matmul(out=psum3[:], lhsT=l2[:], rhs=W3sb[:], start=True, stop=True)

    # ---------------- softmax (rows 0..43 only, fp32) ----------------
    logits = wpool.tile([NROI, NUM_CLASSES], F32, name="logits")
    V.tensor_tensor(out=logits[:], in0=psum3[0:NROI, :], in1=b3sb[:], op=OP.add)
    mxn = wpool.tile([NROI, 1], F32, name="mxn")
    V.tensor_reduce(out=mxn[:], in_=logits[:], axis=AX_X, op=OP.max, negate=True)
    ex = wpool.tile([NROI, NUM_CLASSES], F32, name="ex")
    nc.scalar.activation(out=ex[:], in_=logits[:], func=AF.Exp, bias=mxn[:, 0:1], scale=1.0)
    ssum = wpool.tile([NROI, 1], F32, name="ssum")
    V.tensor_reduce(out=ssum[:], in_=ex[:], axis=AX_X, op=OP.add)
    rinv = wpool.tile([NROI, 1], F32, name="rinv")
    V.reciprocal(rinv[:], ssum[:])
    probs = wpool.tile([NROI, NUM_CLASSES], F32, name="probs")
    V.tensor_scalar(out=probs[:], in0=ex[:], scalar1=rinv[:, 0:1], scalar2=None, op0=OP.mult)

    nc.sync.dma_start(out.rearrange("b p c -> (b p) c"), probs[:])


def build_module():
    nc = bacc.Bacc(get_trn_type() or "TRN2", target_bir_lowering=False, debug=False)
    fm = nc.dram_tensor("feature_map", [B_LOC, H, W, C], F32, kind="ExternalInput")
    prop = nc.dram_tensor("proposals", [B_LOC, P, 4], F32, kind="ExternalInput")
    W1 = nc.dram_tensor("W1", [4096, HID1], F32, kind="ExternalInput")
    b1 = nc.dram_tensor("b1", [HID1], F32, kind="ExternalInput")
    W2 = nc.dram_tensor("W2", [HID1, HID2], F32, kind="ExternalInput")
    b2 = nc.dram_tensor("b2", [HID2], F32, kind="ExternalInput")
    W3 = nc.dram_tensor("W3", [HID2, NUM_CLASSES], F32, kind="ExternalInput")
    b3 = nc.dram_tensor("b3", [NUM_CLASSES], F32, kind="ExternalInput")
    out = nc.dram_tensor("out", [B_LOC, P, NUM_CLASSES], F32, kind="ExternalOutput")

    ident_np, cb32_np, cidx_np = _static_consts()
    ident_c = nc.inline_tensor(ident_np, name="c_ident")
    cb32_c = nc.inline_tensor(cb32_np, name="c_cb32")
    cidx_c = nc.inline_tensor(cidx_np, name="c_cidx")

    with tile.TileContext(nc) as tc:
        emit_kernel(nc, tc, fm[:], prop[:], W1[:], b1[:], W2[:], b2[:], W3[:], b3[:],
                    out[:], (ident_c[:], cb32_c[:], cidx_c[:]))
    nc.compile()
    return nc


_NC_CACHE = None


def _get_module():
    global _NC_CACHE
    if _NC_CACHE is None:
        _NC_CACHE = build_module()
    return _NC_CACHE


def _shard_inputs(inputs):
    f = {k: np.ascontiguousarray(np.asarray(v, dtype=np.float32)) for k, v in inputs.items()}
    in_maps = []
    for c in range(N_CORES):
        sl = slice(B_LOC * c, B_LOC * (c + 1))
        in_maps.append({
            "feature_map": f["feature_map"][sl],
            "proposals": f["proposals"][sl],
            "W1": f["W1"], "b1": f["b1"],
            "W2": f["W2"], "b2": f["b2"],
            "W3": f["W3"], "b3": f["b3"],
        })
    return in_maps


def run(inputs, trace=False):
    """Run on all 8 cores; returns (output [16,22,10], BassKernelResults)."""
    nc = _get_module()
    res = run_bass_kernel_spmd(nc, _shard_inputs(inputs), core_ids=list(range(N_CORES)),
                               trace=trace)
    out = np.concatenate([r["out"] for r in res.results], axis=0)
    return out, res


def kernel(**inputs) -> np.ndarray:
    out, _ = run(inputs, trace=False)
    return out
